# revision 34
# baseline (speedup 1.0000x reference)
"""ANNA sparse attention (B=1, N=2048, C=512, H=8, 64 landmarks, top-4) on
8 TRN2 NeuronCores — one head per core, raw Bass with explicit semaphores.

Per-core graph (head h supplied via per-core weight slices; SPMD-identical):
  - qk projection in full fp32 (exact top-4 routing), v projection in bf16
  - scoresT(j,i) via ONE stacked float32r matmul contracting 128 partitions:
    lhsT=[kT;eT] j-tile, rhs=[qT;maskT] query chunk.  The eT/maskT rows add
    the -1e9 routing mask inside the same matmul (zero extra PE cycles).
  - softmax without max-subtraction (scores are O(1)): exp on ScalarE with
    scale=1/8 -> bf16; AV matmul in bf16 with a ones column giving the
    row-sum for free; normalize via reciprocal + rank-1 fp32 matmul
    broadcast of 1/rowsum across partitions.
  - top-4 per query on VectorE (3x group-max removal + threshold), per
    512-query chunk, pipelined under the PE scores/AV matmuls.
  - AllToAll exchanges head outputs; each core projects its own 256-token
    slice with the full w_proj (float32r) + bias.

Host side only shards/reshapes/concatenates — all FLOPs are on device.
"""

from contextlib import ExitStack

import numpy as np

import concourse.bass as bass
from concourse import mybir
from concourse.bass_utils import run_bass_kernel_spmd

F32 = mybir.dt.float32
F32R = mybir.dt.float32r
BF16 = mybir.dt.bfloat16
AF = mybir.ActivationFunctionType
OP = mybir.AluOpType
AX = mybir.AxisListType

H = 8          # heads == cores
N = 2048       # tokens
C = 512        # model dim
DH = 64        # head dim
S = 64         # landmark segments
SEG = 32       # tokens per segment
KL = 4         # top-k segments
NT = N // 128  # 16 j-tiles of 128 keys
NCH = 4        # query chunks of 512
TOK = N // H   # 256 tokens of final output per core
NEG = -1.0e9
SCALE = DH ** -0.5


def build_nc(debug=False):
    nc = bass.Bass()

    xT_d = nc.declare_dram_parameter("xT", [C, N], F32, isOutput=False)
    wqk_d = nc.declare_dram_parameter("wqkT", [C, 128], F32, isOutput=False)
    wv_d = nc.declare_dram_parameter("wvT", [C, DH], F32, isOutput=False)
    wp_d = nc.declare_dram_parameter("wpT", [C, C], F32, isOutput=False)
    bR_d = nc.declare_dram_parameter("bR", [128, 4], F32, isOutput=False)
    out_d = nc.declare_dram_parameter("out", [C, TOK], F32, isOutput=True)
    dbg = {}
    if debug:
        for nm, shape in [("d_qm", [128, N]), ("d_ke", [128, N]),
                          ("d_cent", [DH, S]), ("d_rt", [128, 256]),
                          ("d_ge", [128, 256]), ("d_vT", [DH, N]),
                          ("d_outTn", [DH, N]), ("d_gsb", [128, 4 * TOK])]:
            dbg[nm] = nc.declare_dram_parameter(nm, shape, F32, isOutput=True)

    eT_np = np.repeat(np.eye(S, dtype=np.float32), SEG, axis=1)  # (64, 2048)
    eT_d = nc.inline_tensor(eT_np, "eT")
    id_d = nc.inline_tensor(np.eye(128, dtype=np.float32), "ident_d")
    ones_d = nc.inline_tensor(np.ones((1, DH), np.float32), "ones64_d")

    # two half-AllToAlls over tokens [h*1024,(h+1)*1024): dest d gets
    # tokens [h*1024 + d*128, +128) from every head
    a2a_in = [nc.dram_tensor(f"a2a_in{h}", [C, TOK // 2], F32) for h in range(2)]
    a2a_out = [nc.dram_tensor(f"a2a_out{h}", [C, TOK // 2], F32) for h in range(2)]

    with ExitStack() as ctx:
        def sb(name, shape, dtype=F32):
            return ctx.enter_context(nc.sbuf_tensor(name, shape, dtype))

        def psum(name, shape):
            return ctx.enter_context(nc.psum_tensor(name, shape, F32))

        def sem(name):
            return ctx.enter_context(nc.semaphore(name))

        # ---- SBUF ----
        ident = sb("ident", [128, 128])
        ones64 = sb("ones64", [1, DH])
        bR = sb("bR_s", [128, 4])
        eS = sb("eS", [S, N])
        wqk = sb("wqk", [128, 512])
        wv = sb("wv", [128, 4 * DH])
        wp = sb("wp", [128, 4 * C])
        wpb = sb("wpb", [128, 4 * C], BF16)
        wvb = sb("wvb", [128, 4 * DH], BF16)
        xsb = sb("xsb", [128, 4 * N])          # chunk kc at cols [kc*N, (kc+1)*N)
        xb = sb("xb", [128, 4 * N], BF16)
        ke = sb("ke", [128, N])                # rows 0:64 kT (fp32, for centroids)
        keb = sb("keb", [128, N], BF16)        # rows 0:64 kT, 64:128 eT (bf16)
        qm = sb("qm", [128, N])                # rows 0:64 qT (fp32, for routing)
        qmb = sb("qmb", [128, N], BF16)        # rows 0:64 qT, 64:128 maskT (bf16)
        vT = sb("vT", [DH, N])
        v1 = sb("v1", [128, NT * (DH + 1)], BF16)
        centT = sb("centT", [DH, S])
        rt = sb("rt", [128, 256])
        wk = sb("wk", [128, 256])
        mx = sb("mx", [128, 4])
        ge = sb("ge", [128, 256])
        pT = [sb(f"pT{i}", [128, 512], BF16) for i in range(4)]
        inv = sb("inv", [1, 512])
        onsb = sb("onsb", [DH + 1, 512])
        outTn = sb("outTn", [DH, N])
        gsb = sb("gsb", [128, 4 * TOK])
        gb = sb("gb", [128, 4 * TOK], BF16)
        ob = [sb("ob0", [128, TOK]), sb("ob1", [128, TOK])]

        # ---- PSUM (8 banks) ----
        # psSc: 4-bank rotation shared by qk-proj groups, scores, proj.
        psSc = [psum(f"psS{i}", [128, 512]) for i in range(4)]
        psV, psT = psSc[2], psSc[3]
        psO = psum("psO", [DH + 1, 512])
        psB = psum("psB", [DH, 512])
        psR = psum("psR", [128, 512])
        psM = psum("psM", [S, 512])
        psVB = [psV, psB]       # v-proj double buffer
        psTR = [psT, psR]       # v-transpose double buffer

        # ---- semaphores ----
        s_cst = sem("s_cst")     # const DMAs: ident, ones, bR, eS      -> 64
        s_wqk = sem("s_wqk")     # 4 dmas -> 64
        s_wv = sem("s_wv")       # 4 dmas -> 64
        s_wp = sem("s_wp")       # 4 dmas -> 64
        s_x = [sem(f"s_x{kc}") for kc in range(4)]  # x chunk kc (1 dma) -> 16
        s_peA = sem("s_peA")     # qk t-group done            -> t+1
        s_peR = sem("s_peR")     # route matmul m done        -> m+1
        s_peV = sem("s_peV")     # v t-group done             -> t+1
        s_peT = sem("s_peT")     # v transpose jt done        -> jt+1
        s_peM = sem("s_peM")     # mask transpose m done      -> m+1
        s_pesc = sem("s_pesc")   # scores k done              -> k+1
        s_peO = sem("s_peO")     # AV chunk c done            -> c+1
        s_peB = sem("s_peB")     # bcast c done               -> c+1
        s_peP = sem("s_peP")     # proj mc done               -> mc+1
        s_actA = sem("s_actA")   # qk copies (4 per t-group)  -> 4t+{1..4}
        s_actE = sem("s_actE")   # eS->keb copy               -> 1
        s_actV = sem("s_actV")   # vT copy t                  -> t+1
        s_actM = sem("s_actM")   # mask copy m                -> m+1
        s_exp = sem("s_exp")     # exp k                      -> k+1
        s_onsb = sem("s_onsb")   # onsb copy c                -> c+1
        s_v1 = sem("s_v1")       # v1 copy jt                 -> jt+1
        s_wpb = sem("s_wpb")     # wpb copies                 -> 4
        s_gb = sem("s_gb")       # gb copies                  -> 4
        s_ob = sem("s_ob")       # bias+copy mc               -> mc+1
        s_xb = sem("s_xb")       # wvb + xb casts             -> 5
        s_cent = sem("s_cent")   # centT partial t            -> t+1
        s_rtc = sem("s_rtc")     # rt copy m                  -> m+1
        s_msk = sem("s_msk")     # top4 chunk c done          -> c+1
        s_inv = sem("s_inv")     # reciprocal c               -> c+1
        s_norm = sem("s_norm")   # normalize c                -> c+1
        s_a2i = [sem("s_a2i0"), sem("s_a2i1")]   # staging dmas/half -> 128
        s_cc = sem("s_cc")       # collectives                -> h+1
        s_gsb = [sem("s_gsb0"), sem("s_gsb1")]   # gsb dmas/half -> 64
        s_out = [sem("s_out0"), sem("s_out1")]   # per-ob-buffer out dmas
        s_dbg = sem("s_dbg")
        s_t4 = sem("s_t4")       # intra-DVE top4 chain ordering
        s_vms = sem("s_vms")     # v1 memset done

        with nc.Block() as block:

            # ================= SYNC: weights/const DMA + staging/out =================
            @block.sync
            def _(sync):
                for kc in range(4):
                    sync.dma_start(out=wqk[:, kc * 128:(kc + 1) * 128],
                                   in_=wqk_d[kc * 128:(kc + 1) * 128, :]).then_inc(s_wqk, 16)
                sync.dma_start(out=xsb[:, 3 * N:4 * N],
                               in_=xT_d[384:512, :]).then_inc(s_x[3], 16)
                for kc in range(4):
                    sync.dma_start(out=wv[:, kc * DH:(kc + 1) * DH],
                                   in_=wv_d[kc * 128:(kc + 1) * 128, :]).then_inc(s_wv, 16)
                sync.dma_start(out=ident[:], in_=id_d[:]).then_inc(s_cst, 16)
                sync.dma_start(out=ones64[:], in_=ones_d[:]).then_inc(s_cst, 16)
                sync.dma_start(out=bR[:], in_=bR_d[:]).then_inc(s_cst, 16)
                sync.dma_start(out=eS[:], in_=eT_d[:]).then_inc(s_cst, 16)
                for kc in range(4):
                    sync.dma_start(out=wp[:, kc * C:(kc + 1) * C],
                                   in_=wp_d[kc * 128:(kc + 1) * 128, :]).then_inc(s_wp, 16)
                # AllToAll staging: chunk c covers dests 4*(c%2)+j of half c//2
                def staging(c):
                    h = c // 2
                    sync.wait_ge(s_norm, c + 1)
                    for j in range(4):
                        d = 4 * (c % 2) + j
                        sync.dma_start(
                            out=a2a_in[h][d * DH:(d + 1) * DH, :],
                            in_=outTn[:, c * 512 + j * 128: c * 512 + (j + 1) * 128],
                        ).then_inc(s_a2i[h], 16)

                def gsb_dmas(h):
                    sync.wait_ge(s_cc, h + 1)
                    for kc in range(4):
                        sync.dma_start(
                            out=gsb[:, kc * TOK + h * 128: kc * TOK + (h + 1) * 128],
                            in_=a2a_out[h][kc * 128:(kc + 1) * 128, :],
                        ).then_inc(s_gsb[h], 16)

                staging(0)
                staging(1)
                staging(2)
                staging(3)
                gsb_dmas(1)
                for mc in range(4):
                    sync.wait_ge(s_ob, mc + 1)
                    sync.dma_start(out=out_d[mc * 128:(mc + 1) * 128, :],
                                   in_=ob[mc % 2][:]).then_inc(s_out[mc % 2], 16)
                if debug:
                    sync.wait_ge(s_norm, 4)
                    n_dbg = 0
                    for nm, src_ap in [("d_qm", qm[:]), ("d_ke", ke[:]),
                                       ("d_cent", centT[:]), ("d_rt", rt[:]),
                                       ("d_ge", ge[:]), ("d_vT", vT[:]),
                                       ("d_outTn", outTn[:]), ("d_gsb", gsb[:])]:
                        sync.dma_start(out=dbg[nm][:], in_=src_ap).then_inc(s_dbg, 16)
                        n_dbg += 1
                    sync.wait_ge(s_dbg, 16 * n_dbg)
                sync.wait_ge(s_out[0], 32)
                sync.wait_ge(s_out[1], 32)

            # ================= GPSIMD: x DMA + collective =================
            @block.gpsimd
            def _(g):
                g.dma_start(out=xsb[:, 0:N],
                            in_=xT_d[0:128, :]).then_inc(s_x[0], 16)
                for h in range(2):
                    g.wait_ge(s_a2i[h], 128)
                    g.collective_compute(
                        "AllToAll", OP.bypass,
                        replica_groups=[list(range(H))],
                        ins=[a2a_in[h][:]],
                        outs=[a2a_out[h][:]],
                    ).then_inc(s_cc)
                    if h == 0:
                        # read back A2A#1 immediately, off the sync queue
                        g.wait_ge(s_cc, 1)
                        for kc in range(4):
                            g.dma_start(
                                out=gsb[:, kc * TOK: kc * TOK + 128],
                                in_=a2a_out[0][kc * 128:(kc + 1) * 128, :],
                            ).then_inc(s_gsb[0], 16)

            # ================= PE (tensor) =================
            @block.tensor
            def _(pe):
                # qk projection, full fp32: group t -> psSc[t] (no bank reuse)
                pe.wait_ge(s_wqk, 64)
                for kc in range(4):
                    pe.wait_ge(s_x[kc], 16)
                    for t in range(4):
                        inst = pe.matmul(
                            psSc[t][:],
                            wqk[:, kc * 128:(kc + 1) * 128],
                            xsb[:, kc * N + t * 512: kc * N + (t + 1) * 512],
                            start=(kc == 0), stop=(kc == 3))
                        if kc == 3:
                            inst.then_inc(s_peA)

                # route chunk helper (fp32)
                pe.wait_ge(s_cent, 4)

                def route_chunk(c):
                    for it in range(4):
                        m = c * 4 + it
                        if m >= 1:
                            pe.wait_ge(s_rtc, m)
                        pe.matmul(
                            psR[:, 0:S],
                            qm[0:DH, c * 512 + it * 128: c * 512 + (it + 1) * 128],
                            centT[:], start=True, stop=True).then_inc(s_peR)

                route_chunk(0)

                # v projection, bf16: group t -> psVB[t%2]
                pe.wait_ge(s_xb, 5)
                pe.wait_ge(s_actA, 12)   # psV == psSc[2]: qk group 2 drained
                for t in range(4):
                    if t >= 2:
                        pe.wait_ge(s_actV, t - 1)
                    for kc in range(4):
                        inst = pe.matmul(
                            psVB[t % 2][0:DH, :],
                            wvb[:, kc * DH:(kc + 1) * DH],
                            xb[:, kc * N + t * 512: kc * N + (t + 1) * 512],
                            start=(kc == 0), stop=(kc == 3))
                        if kc == 3:
                            inst.then_inc(s_peV)

                # v transposes (fp32), alternating psT/psR
                pe.wait_ge(s_cst, 64)
                pe.wait_ge(s_actA, 16)   # psT == psSc[3]: qk group 3 drained
                pe.wait_ge(s_rtc, 4)     # psR: route(0) copies drained
                for jt in range(NT):
                    if jt >= 2:
                        pe.wait_ge(s_v1, jt - 1)
                    pe.transpose(psTR[jt % 2][:, 0:DH], vT[:, jt * 128:(jt + 1) * 128],
                                 ident[0:DH, 0:DH]).then_inc(s_peT)

                # per query chunk: mask transposes, route(c+1), scores+AV, bcast
                for cch in range(NCH):
                    pe.wait_ge(s_msk, cch + 1)
                    for it in range(4):
                        m = cch * 4 + it
                        if m >= 1:
                            pe.wait_ge(s_actM, m)
                        pe.transpose(psM[:, 0:128], ge[:, it * DH:(it + 1) * DH],
                                     ident[:]).then_inc(s_peM)
                    if cch + 1 < NCH:
                        route_chunk(cch + 1)
                    pe.wait_ge(s_actM, 4 * (cch + 1))
                    if cch == 0:
                        pe.wait_ge(s_actE, 1)
                        pe.wait_ge(s_actV, 4)   # psV drained by vT copies
                        pe.wait_ge(s_v1, 16)    # psT/psR drained by v1 copies

                    def avmm(lk):
                        k = cch * NT + lk
                        pe.wait_ge(s_exp, k + 1)
                        if lk == 0 and cch > 0:
                            pe.wait_ge(s_onsb, cch)
                        inst = pe.matmul(psO[:],
                                         v1[:, lk * (DH + 1):(lk + 1) * (DH + 1)],
                                         pT[k % 4][:],
                                         start=(lk == 0), stop=(lk == NT - 1),
                                         skip_group_check=True)
                        if lk == NT - 1:
                            inst.then_inc(s_peO)

                    for lk in range(NT):
                        k = cch * NT + lk
                        # bank psSc[k%4] drain (s_exp >= k-3) is dominated by
                        # the previous avmm's s_exp wait on the in-order PE
                        pe.matmul(psSc[k % 4][:],
                                  keb[:, lk * 128:(lk + 1) * 128],
                                  qmb[:, cch * 512:(cch + 1) * 512],
                                  start=True, stop=True,
                                  skip_group_check=True).then_inc(s_pesc)
                        if lk == 2 and cch >= 1:
                            # bcast of previous chunk, off the critical path
                            pe.wait_ge(s_inv, cch)
                            if cch >= 2:
                                pe.wait_ge(s_norm, cch - 1)
                            pe.matmul(psB[:], ones64[:], inv[:],
                                      start=True, stop=True).then_inc(s_peB)
                        if lk >= 3:
                            avmm(lk - 3)
                    for lk in range(NT - 3, NT):
                        avmm(lk)
                # bcast of the final chunk
                pe.wait_ge(s_inv, NCH)
                pe.wait_ge(s_norm, NCH - 1)
                pe.matmul(psB[:], ones64[:], inv[:],
                          start=True, stop=True).then_inc(s_peB)

                # output projection (bf16): (half h, mc) -> psSc[mc][:, h*128:]
                pe.wait_ge(s_wpb, 4)
                for h in range(2):
                    pe.wait_ge(s_gb, 4 * (h + 1))
                    for mc in range(4):
                        if h == 0:
                            pe.wait_ge(s_exp, 61 + mc)   # bank drained by exp
                        for kc in range(4):
                            inst = pe.matmul(
                                psSc[mc][:, h * 128:(h + 1) * 128],
                                wpb[:, kc * C + mc * 128: kc * C + (mc + 1) * 128],
                                gb[:, kc * TOK + h * 128: kc * TOK + (h + 1) * 128],
                                start=(kc == 0), stop=(kc == 3),
                                skip_group_check=True)
                            if kc == 3:
                                inst.then_inc(s_peP)

            # ================= ACT (scalar) =================
            @block.scalar
            def _(act):
                act.dma_start(out=xsb[:, N:2 * N],
                              in_=xT_d[128:256, :]).then_inc(s_x[1], 16)
                act.dma_start(out=xsb[:, 2 * N:3 * N],
                              in_=xT_d[256:384, :]).then_inc(s_x[2], 16)
                # qk copies: psSc[t] rows -> qm/qmb (q) and ke/keb (k)
                for t in range(4):
                    act.wait_ge(s_peA, t + 1)
                    cols = slice(t * 512, (t + 1) * 512)
                    act.activation(qm[0:DH, cols], psSc[t][0:DH, :],
                                   AF.Copy).then_inc(s_actA)
                    act.activation(qmb[0:DH, cols], psSc[t][0:DH, :],
                                   AF.Copy).then_inc(s_actA)
                    act.activation(ke[0:DH, cols], psSc[t][DH:128, :],
                                   AF.Copy).then_inc(s_actA)
                    act.activation(keb[0:DH, cols], psSc[t][DH:128, :],
                                   AF.Copy).then_inc(s_actA)
                act.wait_ge(s_cst, 64)
                act.activation(keb[DH:128, :], eS[:], AF.Copy).then_inc(s_actE)
                act.wait_ge(s_wp, 64)
                for kc in range(4):
                    act.activation(wpb[:, kc * C:(kc + 1) * C],
                                   wp[:, kc * C:(kc + 1) * C], AF.Copy).then_inc(s_wpb)
                for t in range(4):
                    act.wait_ge(s_peV, t + 1)
                    act.activation(vT[:, t * 512:(t + 1) * 512], psVB[t % 2][0:DH, :],
                                   AF.Copy).then_inc(s_actV)
                act.wait_ge(s_vms, 1)
                for jt in range(NT):
                    act.wait_ge(s_peT, jt + 1)
                    act.activation(v1[:, jt * (DH + 1): jt * (DH + 1) + DH],
                                   psTR[jt % 2][:, 0:DH], AF.Copy).then_inc(s_v1)
                for cch in range(NCH):
                    for it in range(4):
                        m = cch * 4 + it
                        act.wait_ge(s_peM, m + 1)
                        act.activation(
                            qmb[DH:128, cch * 512 + it * 128: cch * 512 + (it + 1) * 128],
                            psM[:, 0:128], AF.Copy).then_inc(s_actM)
                    for lk in range(NT):
                        k = cch * NT + lk
                        act.wait_ge(s_pesc, k + 1)
                        act.activation(pT[k % 4][:], psSc[k % 4][:],
                                       AF.Exp, scale=SCALE).then_inc(s_exp)
                    act.wait_ge(s_peO, cch + 1)
                    if cch >= 1:
                        act.wait_ge(s_norm, cch)  # DVE done reading onsb(cch-1)
                    act.activation(onsb[:], psO[0:DH + 1, :], AF.Copy).then_inc(s_onsb)
                for mc in range(4):
                    act.wait_ge(s_peP, 5 + mc)   # both halves of bank mc done
                    if mc >= 2:
                        act.wait_ge(s_out[mc % 2], 16)  # out DMA done with ob
                    act.activation(ob[mc % 2][:], psSc[mc][:, 0:TOK],
                                   AF.Identity, bias=bR[:, mc:mc + 1]).then_inc(s_ob)

            # ================= DVE (vector) =================
            @block.vector
            def _(dve):
                dve.wait_ge(s_wv, 64)
                dve.tensor_copy(wvb[:], wv[:]).then_inc(s_xb)
                for kc in range(4):
                    dve.wait_ge(s_x[kc], 16)
                    dve.tensor_copy(xb[:, kc * N:(kc + 1) * N],
                                    xsb[:, kc * N:(kc + 1) * N]).then_inc(s_xb)
                dve.memset(v1[:], 1.0).then_inc(s_vms)
                # centroids: per token-chunk t, segments t*16..t*16+16
                for t in range(4):
                    dve.wait_ge(s_actA, 4 * t + 3)
                    dve.tensor_reduce(
                        centT[:, t * 16:(t + 1) * 16],
                        ke[0:DH, t * 512:(t + 1) * 512].rearrange(
                            "p (s g) -> p s g", g=SEG),
                        AX.X, OP.add).then_inc(s_cent)

                t4n = [0]

                def chain(inst):
                    inst.then_inc(s_t4)
                    t4n[0] += 1
                    dve.wait_ge(s_t4, t4n[0])

                def rtcopies(cch):
                    if t4n[0] > 0:
                        # prior top4 chain fully drained before rt overwrite
                        dve.wait_ge(s_t4, t4n[0])
                    for it in range(4):
                        m = cch * 4 + it
                        dve.wait_ge(s_peR, m + 1)
                        dve.tensor_copy(rt[:, it * DH:(it + 1) * DH],
                                        psR[:, 0:S]).then_inc(s_rtc)

                def top4(cch):
                    dve.wait_ge(s_rtc, 4 * cch + 4)
                    if cch >= 1:
                        # PE mask transposes of chunk cch-1 still read ge
                        dve.wait_ge(s_peM, 4 * cch)
                    wv3 = wk[:].rearrange("p (g s) -> p g s", s=S)
                    rv3 = rt[:].rearrange("p (g s) -> p g s", s=S)
                    gv3 = ge[:].rearrange("p (g s) -> p g s", s=S)
                    mxb = mx[:].broadcast_to([128, 4, S])
                    chain(dve.tensor_copy(wk[:], rt[:]))
                    for _i in range(KL - 1):
                        chain(dve.tensor_reduce(mx[:], wv3, AX.X, OP.max))
                        chain(dve.tensor_tensor(gv3, wv3, mxb, OP.is_ge))
                        chain(dve.scalar_tensor_tensor(wv3, gv3, NEG, wv3,
                                                       OP.mult, OP.add))
                    chain(dve.tensor_reduce(mx[:], wv3, AX.X, OP.max))
                    chain(dve.tensor_tensor(gv3, rv3, mxb, OP.is_lt))
                    dve.tensor_scalar(ge[:], ge[:], NEG, None,
                                      OP.mult).then_inc(s_msk)

                rtcopies(0)
                top4(0)
                rtcopies(1)
                for cch in range(NCH):
                    if cch + 1 < NCH:
                        top4(cch + 1)   # overlaps PE scores/AV of chunk cch
                    dve.wait_ge(s_onsb, cch + 1)
                    dve.reciprocal(inv[:], onsb[DH:DH + 1, :]).then_inc(s_inv)
                    if cch + 2 < NCH:
                        rtcopies(cch + 2)   # before norm(cch): PE route(cch+2) ping-pong
                    dve.wait_ge(s_peB, cch + 1)
                    dve.tensor_tensor(outTn[:, cch * 512:(cch + 1) * 512],
                                      onsb[0:DH, :], psB[:], OP.mult).then_inc(s_norm)
                    if cch == 2:
                        dve.wait_ge(s_gsb[0], 64)
                        for kc in range(4):
                            dve.tensor_copy(
                                gb[:, kc * TOK: kc * TOK + 128],
                                gsb[:, kc * TOK: kc * TOK + 128]).then_inc(s_gb)
                    if cch == 3:
                        dve.wait_ge(s_gsb[1], 64)
                        for kc in range(4):
                            dve.tensor_copy(
                                gb[:, kc * TOK + 128: kc * TOK + 256],
                                gsb[:, kc * TOK + 128: kc * TOK + 256]).then_inc(s_gb)

    return nc


def prep_in_maps(x, w_qkv, w_proj, b_proj):
    x = np.asarray(x, np.float32)
    w_qkv = np.asarray(w_qkv, np.float32)
    w_proj = np.asarray(w_proj, np.float32)
    b_proj = np.asarray(b_proj, np.float32)
    assert x.shape == (1, N, C) and w_qkv.shape == (3 * C, C)

    xT = np.ascontiguousarray(x[0].T)                      # (512, 2048)
    wpT = np.ascontiguousarray(w_proj.T)                   # (512, 512) d_global x c_out
    bR = np.ascontiguousarray(b_proj.reshape(4, 128).T)    # (128, 4)

    in_maps = []
    for h in range(H):
        wq = w_qkv[h * DH:(h + 1) * DH, :]                 # (64, 512)
        wkk = w_qkv[C + h * DH: C + (h + 1) * DH, :]
        wvh = w_qkv[2 * C + h * DH: 2 * C + (h + 1) * DH, :]
        wqkT = np.ascontiguousarray(np.concatenate([wq, wkk], 0).T)  # (512, 128)
        wvT = np.ascontiguousarray(wvh.T)                  # (512, 64)
        in_maps.append({
            "xT": xT, "wqkT": wqkT, "wvT": wvT, "wpT": wpT, "bR": bR,
        })
    return in_maps


def assemble(outs):
    """Core c's (512, 256) slice covers tokens [c*128,(c+1)*128) and
    [1024+c*128, 1024+(c+1)*128)."""
    full = np.empty((C, N), np.float32)
    for c in range(H):
        full[:, c * 128:(c + 1) * 128] = outs[c][:, 0:128]
        full[:, 1024 + c * 128: 1024 + (c + 1) * 128] = outs[c][:, 128:256]
    return np.ascontiguousarray(full.T).reshape(1, N, C).astype(np.float32)


_NC_CACHE = {}


def get_nc():
    if "nc" not in _NC_CACHE:
        _NC_CACHE["nc"] = build_nc()
    return _NC_CACHE["nc"]


def kernel(x, w_qkv, w_proj, b_proj, _trace=False):
    nc = get_nc()
    in_maps = prep_in_maps(x, w_qkv, w_proj, b_proj)
    res = run_bass_kernel_spmd(nc, in_maps, core_ids=list(range(H)), trace=_trace)
    outs = [np.asarray(res.results[i]["out"]) for i in range(H)]  # (512, 256) each
    out = assemble(outs)
    if _trace:
        return out, res
    return out
